# revision 15
# baseline (speedup 1.0000x reference)
"""Trainium2 Bass kernel for nn_GCN_26242250179008.

The reference model is a ChebConv(K=1) stack, which degenerates to plain
dense linear layers (edge_index is never used):

    h = relu(x @ W1.T + b1); h = relu(h @ W2.T + b2); h = h @ W3.T + b3
    g = mean(h, axis=0); out = log_softmax(g @ Wl.T + bl)

Because layer 3 is linear (no relu), mean() commutes with it:
    mean(h3) = mean(h2) @ W3.T + b3
so the device only needs sum_n relu(W2 @ relu(W1 @ x_n + b1) + b2) — a
[128] partial per core.  Layer 3, the classifier head and log_softmax are
O(128^2) and run on host.

Sharding: data-parallel over nodes.  x is split row-wise into 8 shards of
25000 nodes; each shard is transposed on host to [128 features, 25000
nodes] so features sit on SBUF partitions and the matmuls need no
on-device transpose.

Precision/scaling: layer 1 runs in fp8 e4m3 (x pre-scaled by 32, W1 by
16, both exactly representable ranges; biases by 512), layer 2 in fp16;
relu is positively homogeneous so the 512x scale flows through both
layers and the host divides the partial sums by 512 at the end.  fp8
halves the x DMA bytes (3.2MB/core), which removes the DMA-ramp
starvation entirely.  Host-simulated pipeline error vs the fp32
reference: ~7e-5 max-rel (gate is 2e-2).

Correctness notes (HW-verified):
  * The DVE tensor_scalar/scalar_tensor_tensor accumulator reduces along
    the free dim with **op1**, not with a fixed sum (the bass docstring
    says "sum"; TRN2 HW disagrees).  A relu written as (ps+b) max 0
    accumulates the column MAX.  relu2 on DVE instead uses the identity
    relu(z+b) = max(z, -b) + b  →  (ps max -b2) add b2bcast, so the
    trailing op is `add` and the accumulator is a true sum.
  * tensor_scalar with BOTH scalar operands as [128,1] APs mis-lowers on
    HW (garbage second scalar); scalar_tensor_tensor with a stride-0
    broadcast AP as in1 is used instead.
  * The ScalarE ACTIVATE accumulator always sums.

Perf notes (HW-measured):
  * Steady state is bound by PSUM evacuation, split across the two
    PSUM-capable engines (GPSIMD has no PSUM port; DVE perf modes need
    16-bit operands so fp32-PSUM reads are stuck at 1x on both engines).
    ScalarE ACTIVATE(relu) and DVE ops each stream ~1 col/cycle (1.2 /
    0.96 GHz); the planner below balances groups across the two engines
    using issue-spacing costs.
  * Preamble: the framework entry barrier + register loads take ~6.8us
    before any user op.  The ACT relu table is pre-loaded via a dummy
    activation (~2.7us ACT_TABLE_LOAD+DRAIN overlaps the x DMA), 2 dummy
    matmuls on scratch warm the PE HAM clock gate (cold PE runs at
    1.2GHz for ~3.4us of activity), and the first groups' evac runs on
    DVE, which is ready before ACT.  The dummy ops read/write tiles that
    nothing else writes, so the Tile scheduler cannot serialize them
    behind real work (a prior revision lost ~5us to exactly that).
  * x chunks ride the sync HWDGE ring in consumption order with ramping
    sizes.  (A/B'd alternatives that LOST: scalar-ring interleaving —
    big act-ring chunks starve the early sync chunks via packet
    round-robin; gpsimd SWDGE — ~6us/chunk of Q7 descriptor emission.)
  * The per-group sums go out as one [128, 32] fp32 DMA (128B per
    partition).  A [128,1] output emits 4-byte descriptors whose HBM
    read-modify-write completion costs ~8us extra at the end.
"""

import os

import numpy as np

# Ask NRT to reset cores at init: clears wedged/throttled device state
# left by earlier runs (observed ~8us/run slowdown from an accumulated
# power throttle; a reset restored nominal clocks).  No-op when the
# runtime is already initialized or the device is fresh.
os.environ.setdefault("NEURON_RT_RESET_CORES", "1")

N_NODES = 200_000
F = 128
N_CORES = 8
PER_CORE = N_NODES // N_CORES  # 25000
GROUP = 1024
MM_N = 512  # one fp32 PSUM bank per matmul output
# wpack fp16 columns: W1 as fp8 bit-pairs (64) | W2 (128) | biases (8)
W1_COLS = F // 2
WPK_COLS = W1_COLS + F + 8
SCALE = 512.0  # x*32 and W1*16 on host; relu is homogeneous; host /512

# small leading groups so compute starts on the first small DMA chunk;
# a tiny final group shrinks the end-of-pipeline drain
# (relu1 -> mm2 -> relu2 on the last group is serialized)
GROUP_WIDTHS = [256, 512] + [1024] * 23 + [552, 128]
assert sum(GROUP_WIDTHS) == PER_CORE
N_GROUPS = len(GROUP_WIDTHS)  # 27
ACC_COLS = 32  # padded so the out-DMA moves 128B per partition

# x DMA chunks (sync HWDGE ring, consumption order, ramping sizes)
CHUNKS = [1024, 2048, 4096, 4096, 4096, 4096, 4096, 1448]
assert sum(CHUNKS) == PER_CORE

# per-pass issue-spacing costs (ns) for [128,w] fp32 PSUM sources,
# fit from HW traces (spacing, not duration: the ack half of the
# memory-init latency pipelines with the next op)
DVE_NS = lambda w: (w + 132) / 0.96  # incl hidden accum readout
ACT_NS = lambda w: (w + 171) / 1.2  # activation, no accum
ACT_ACC_NS = lambda w: (w + 171) / 1.2 + 180  # activation + accum readout
ACT_LATE_START = 0.0  # both engines end up gated by the first x chunk
DVE_STALL_BIAS = 800.0  # DMA-ramp starvation hits DVE's schedule harder

_COMPILED = {}


def _build_program():
    from concourse import bacc, mybir, tile

    f32 = mybir.dt.float32
    f16 = mybir.dt.float16
    f8 = mybir.dt.float8e4

    nc = bacc.Bacc(None, target_bir_lowering=False, debug=False)

    xt = nc.dram_tensor("xt", [F, PER_CORE], f8, kind="ExternalInput")
    wpk = nc.dram_tensor("wpack", [F, WPK_COLS], f16, kind="ExternalInput")
    out = nc.dram_tensor("partial", [F, ACC_COLS], f32, kind="ExternalOutput")

    # plan the engine split ahead of time with the measured costs:
    # ScalarE prefers relu1 (no accumulator), DVE prefers relu2 (cheap
    # accumulator); swap groups to even the finish times.
    widths = GROUP_WIDTHS
    gstart = [sum(widths[:g]) for g in range(N_GROUPS)]

    relu1_on_dve = set()
    t_act = ACT_LATE_START + sum(ACT_NS(w) for w in widths)
    t_dve = DVE_STALL_BIAS + sum(DVE_NS(w) for w in widths)
    relu2_on_act = set()
    # move relu2 groups (spread through the schedule, excluding the tiny
    # drain group) from DVE to ACT while it improves the makespan
    for g in [21, 13, 19, 7, 23, 10, 16, 5, 12, 17]:
        w = widths[g]
        if max(t_act + ACT_ACC_NS(w), t_dve - DVE_NS(w)) < max(t_act, t_dve):
            t_act += ACT_ACC_NS(w)
            t_dve -= DVE_NS(w)
            relu2_on_act.add(g)
    for g in [6, 14, 20, 11, 24]:
        w = widths[g]
        if max(t_dve + DVE_NS(w), t_act - ACT_NS(w)) < max(t_act, t_dve):
            t_dve += DVE_NS(w)
            t_act -= ACT_NS(w)
            relu1_on_dve.add(g)

    with tile.TileContext(nc, pool_alloc_mode="queue") as tc:
        with (
            tc.tile_pool(name="const", bufs=1) as cpool,
            tc.tile_pool(name="h1", bufs=8) as h1pool,
            tc.tile_pool(name="ps1", bufs=2, space="PSUM") as ps1pool,
            tc.tile_pool(name="ps2", bufs=2, space="PSUM") as ps2pool,
        ):
            wpk_sb = cpool.tile([F, WPK_COLS], f16)
            acc = cpool.tile([F, ACC_COLS], f32)
            scratch = cpool.tile([F, MM_N], f16)
            scratch2 = cpool.tile([F, 8], f16)
            x_all = cpool.tile([F, PER_CORE], f8)

            # wpk rides the scalar HWDGE ring (its descgen is the first op
            # on the ACT queue, ahead of the table preload) so the sync
            # ring's first descgen slot goes to the first x chunk.
            nc.scalar.dma_start(wpk_sb[:], wpk[:])
            pos = 0
            for w in CHUNKS:
                nc.sync.dma_start(x_all[:, pos : pos + w], xt[:, pos : pos + w])
                pos += w

            w1_sb = wpk_sb[:, 0:W1_COLS].bitcast(f8)  # [F, 128] fp8
            w2_sb = wpk_sb[:, W1_COLS : W1_COLS + F]
            # the last fp16 columns carry b1|b2|-b2 fp32 raw bits (x512)
            bias_f32 = wpk_sb[:, W1_COLS + F : W1_COLS + F + 8].bitcast(f32)
            b1_sb = bias_f32[:, 0:1]
            b2_sb = bias_f32[:, 1:2]
            nb2_sb = bias_f32[:, 2:3]

            nc.vector.memset(scratch[:], 0.0)
            nc.vector.memset(acc[:], 0.0)

            # Pre-load the ACT relu spline table during the DMA wait: the
            # first ACTIVATE triggers a ~1.3us table DMA + ~1.4us drain,
            # which would otherwise land on the critical path.  Writes a
            # dedicated tile so nothing serializes behind it.
            nc.scalar.activation(
                scratch2[:],
                scratch[:, 0:8],
                mybir.ActivationFunctionType.Relu,
            )
            # Warm the PE HAM clock gate with dummy matmuls on scratch so
            # the real matmuls don't start at the cold 1.2GHz rate.  The
            # dst tile aliases ps2's buffer rotation; the WAW dep against
            # the (much later) first real mm2 is harmless.
            warm_ps = ps2pool.tile([F, GROUP], f32, tag="ps2", name="warm")
            for _ in range(4):
                nc.tensor.matmul(
                    warm_ps[:, :MM_N],
                    scratch[:, 0:F],
                    scratch[:],
                    start=True,
                    stop=True,
                )

            def dve_relu1(ps, outp, gw):
                # h1 = (ps + b1) max 0; no accumulator, op order free
                nc.vector.tensor_scalar(
                    outp,
                    ps[:, :gw],
                    b1_sb,
                    0.0,
                    op0=mybir.AluOpType.add,
                    op1=mybir.AluOpType.max,
                )

            def dve_relu2(ps, accum, gw):
                # relu(z+b2) = max(z, -b2) + b2: the trailing op is `add`,
                # so the DVE accumulator (which reduces with op1) sums.
                # b2 rides in1 as a stride-0 broadcast (two AP scalars on
                # one tensor_scalar mis-lower on HW).
                nc.vector.scalar_tensor_tensor(
                    ps[:, :gw],
                    ps[:, :gw],
                    nb2_sb,
                    b2_sb.broadcast_to([F, gw]),
                    op0=mybir.AluOpType.max,
                    op1=mybir.AluOpType.add,
                    accum_out=accum,
                )

            def act_relu(ps, bias, outp, accum, gw):
                nc.scalar.activation(
                    outp,
                    ps[:, :gw],
                    mybir.ActivationFunctionType.Relu,
                    bias=bias,
                    accum_out=accum,
                )

            # issue per pair of groups so the PE stream batches stationary
            # weights: mm1(g) mm1(g+1) [W1 once], relu1s, mm2(g) mm2(g+1)
            # [W2 once], relu2s — halves the LDWEIGHTS count.
            def issue_mm(dst, wsb, src, gw):
                for j in range(0, gw, MM_N):
                    jw = min(MM_N, gw - j)
                    nc.tensor.matmul(
                        dst[:, j : j + jw],
                        wsb,
                        src[:, j : j + jw],
                        start=True,
                        stop=True,
                    )

            for g0 in range(0, N_GROUPS, 2):
                pair = [g for g in (g0, g0 + 1) if g < N_GROUPS]
                ps1s, h1s, ps2s = {}, {}, {}
                for g in pair:
                    gw = widths[g]
                    ps1s[g] = ps1pool.tile([F, GROUP], f32, tag="ps1", name=f"ps1_{g}")
                    issue_mm(ps1s[g], w1_sb, x_all[:, gstart[g] : gstart[g] + gw], gw)
                for g in pair:
                    gw = widths[g]
                    h1s[g] = h1pool.tile([F, GROUP], f16, tag="h1", name=f"h1_{g}")
                    if g in relu1_on_dve:
                        dve_relu1(ps1s[g], h1s[g][:, :gw], gw)
                    else:
                        act_relu(ps1s[g], b1_sb, h1s[g][:, :gw], None, gw)
                for g in pair:
                    gw = widths[g]
                    ps2s[g] = ps2pool.tile([F, GROUP], f32, tag="ps2", name=f"ps2_{g}")
                    issue_mm(ps2s[g], w2_sb, h1s[g][:, :gw], gw)
                for g in pair:
                    gw = widths[g]
                    accum = acc[:, g : g + 1]
                    if g in relu2_on_act:
                        # dead main output goes back into the same PSUM tile
                        # in place — ScalarE's PSUM write port is faster than
                        # its SBUF port, and this keeps the late ACT ops off
                        # the SBUF banks.
                        act_relu(ps2s[g], b2_sb, ps2s[g][:, :gw], accum, gw)
                    else:
                        dve_relu2(ps2s[g], accum, gw)

            nc.sync.dma_start(out[:], acc[:])

    nc.compile()
    return nc


def _get_program():
    if "p" not in _COMPILED:
        _COMPILED["p"] = _build_program()
    return _COMPILED["p"]


def _run_on_device(in_maps, **kwargs):
    from concourse.bass_utils import run_bass_kernel_spmd

    nc = _get_program()
    return run_bass_kernel_spmd(nc, in_maps, core_ids=list(range(N_CORES)), **kwargs)


def _make_in_maps(x, W1, b1, W2, b2):
    import ml_dtypes

    f8 = ml_dtypes.float8_e4m3
    f16 = np.float16
    x = np.ascontiguousarray(np.asarray(x, dtype=np.float32)).reshape(N_NODES, F)
    shards = x.reshape(N_CORES, PER_CORE, F)

    w1q = (np.asarray(W1, np.float32).T * 16.0).astype(f8)  # [F, 128] fp8
    w1bits = np.ascontiguousarray(w1q).view(np.uint8).reshape(F, F)
    w1pack = w1bits.view(np.uint16).view(f16)  # [F, 64] fp16-typed fp8 pairs
    w2pack = np.asarray(W2, np.float32).T.astype(f16)  # [F, 128]
    b1f = np.asarray(b1, np.float32) * SCALE
    b2f = np.asarray(b2, np.float32) * SCALE
    bias_block = np.stack([b1f, b2f, -b2f, np.zeros_like(b1f)], axis=1)  # [F,4] f32
    bbits = np.ascontiguousarray(bias_block).view(np.uint16).view(f16)  # [F,8]
    wpack = np.concatenate([w1pack, w2pack, bbits], axis=1)
    assert wpack.shape == (F, WPK_COLS)

    in_maps = []
    for c in range(N_CORES):
        in_maps.append(
            {
                "xt": np.ascontiguousarray(shards[c].T * 32.0).astype(f8),
                "wpack": wpack,
            }
        )
    return in_maps


def _host_head(partials, W3, b3, Wl, bl):
    # partials: [N_CORES, 128, ACC_COLS] fp32 per-group sums of SCALE*h2.
    g = partials[:, :, :N_GROUPS].astype(np.float64).sum(axis=(0, 2)) / (
        SCALE * float(N_NODES)
    )
    z = np.asarray(W3, np.float64) @ g + np.asarray(b3, np.float64)
    logits = np.asarray(Wl, np.float64) @ z + np.asarray(bl, np.float64)
    m = logits.max()
    ls = logits - (m + np.log(np.exp(logits - m).sum()))
    return ls[None, :].astype(np.float32)


def kernel(x, edge_index, W1, b1, W2, b2, W3, b3, Wl, bl, **_unused):
    # edge_index is unused by the reference computation (ChebConv K=1).
    in_maps = _make_in_maps(x, W1, b1, W2, b2)
    res = _run_on_device(in_maps)
    partials = np.stack(
        [np.asarray(r["partial"], np.float32) for r in res.results]
    )
    return _host_head(partials, W3, b3, Wl, bl)
